# revision 18
# baseline (speedup 1.0000x reference)
"""Trainium2 Bass kernel for nn_DQNDecision (64-step GNN scan).

Self-contained: hardcodes shapes. kernel(**inputs) -> [4096, 64] int16.

Strategy (see DESIGN.md): data-parallel over queries (512/core x 8 cores).
Host fuses masks+bh2 into an additive-mask table TM = [(mask-1)*1e9+bh2, task]
([Q,64,384] f32), precomputes gather offsets from topologicals. Device runs
the 64-step scan: indirect-gather node rows, PE-transpose into matmul layout,
fp32 MLP chain (weights stationary, activations as moving operand, final layer
flipped to produce query-major qv), masked argmax via reduce/is_equal, one-hot
service-feature extraction, carry updates, qos scatter via copy_predicated.
Device outputs (64 - argmax_index) per (query, step); host rebuilds ret.
"""

import os
import numpy as np

P = 128          # partitions
B = 4            # query blocks per core
QL = P * B       # queries per core
NC = 8           # cores
Q = QL * NC      # 4096
NSTEP = 64
S = 64           # services
ND = 320         # task feature width
BW = 456         # gather-tile block width in f32 units:
                 # [0:64 M f32][f16 units 128:512 th(320 task+4 const+60 pad)]
                 # [f16 units 512:896 tl likewise][pad]; host row = 448 f32 (1792B)
GW = B * BW      # gather tile free size
NG = 5           # gather buffer depth (prefetch)
# column offsets within a block of the gather tile
C_M = 0          # additive mask+bias (64 f32)
# f16-unit offsets within a block (block stride = 912 f16 units)
F_TH = 128       # th task (320 f16) + const at 448:452 + pad
F_TL = 512       # tl likewise
BW16 = 912

_cached = {}


def _v(tile_ap, off, dims):
    """Custom free-dim view of a tile AP: dims = [[step, count], ...] (elements)."""
    import concourse.bass as bass
    return bass.AP(tile_ap.tensor, tile_ap.offset + off, [tile_ap.ap[0]] + dims)


def build_program():
    import concourse.bacc as bacc
    import concourse.mybir as mybir
    from concourse.tile import TileContext
    from concourse.masks import make_identity
    from concourse.bass import IndirectOffsetOnAxis

    f32 = mybir.dt.float32
    f16 = mybir.dt.float16
    i32 = mybir.dt.int32
    AOp = mybir.AluOpType
    AF = mybir.ActivationFunctionType
    AX = mybir.AxisListType

    nc = bacc.Bacc(
        "TRN2", target_bir_lowering=False, debug=False,
        enable_asserts=False, num_devices=NC,
    )

    # ---- DRAM IO (per-core shard) ----
    tm_d = nc.dram_tensor("tm", [QL * 64, 448], f32, kind="ExternalInput")
    offs_d = nc.dram_tensor("offs", [P, NSTEP * B], i32, kind="ExternalInput")
    oht_d = nc.dram_tensor("ohtab", [QL * NSTEP * S], mybir.dt.int8, kind="ExternalInput")
    w1_d = nc.dram_tensor("w1", [328, 128], f32, kind="ExternalInput")
    w2_d = nc.dram_tensor("w2", [128, 128], f32, kind="ExternalInput")
    wh1_d = nc.dram_tensor("wh1", [128, 128], f32, kind="ExternalInput")
    wh2_d = nc.dram_tensor("wh2", [128, 64], f32, kind="ExternalInput")
    b1_d = nc.dram_tensor("b1", [128], f32, kind="ExternalInput")
    b2_d = nc.dram_tensor("b2", [128], f32, kind="ExternalInput")
    bh1_d = nc.dram_tensor("bh1", [128], f32, kind="ExternalInput")
    sero_d = nc.dram_tensor("sero", [P, B * NSTEP], f32, kind="ExternalOutput")

    with TileContext(nc) as tc:
        with (
            tc.tile_pool(name="pers", bufs=1) as pp,
            tc.tile_pool(name="work", bufs=2) as wp,
            tc.tile_pool(name="ps_ch", bufs=1, space="PSUM") as pch,
            tc.tile_pool(name="ps_mlp", bufs=1, space="PSUM") as pml,
            tc.tile_pool(name="ps_qv", bufs=1, space="PSUM") as pqv,
        ):
            # ---- persistent tiles ----
            G = [pp.tile([P, GW], f32, tag=f"G{k}", name=f"G{k}") for k in range(NG)]
            qos = pp.tile([P, B * S], f32, tag="qos")
            C = [pp.tile([P, 16], f32, tag=f"C{j}", name=f"C{j}") for j in range(2)]
            offs_sb = pp.tile([P, B * NSTEP], i32, tag="offs")
            oht_sb = pp.tile([P, B * NSTEP * S], mybir.dt.int8, tag="oht_sb")
            riota_i = pp.tile([P, B * S], i32, tag="riota_i")
            riota = pp.tile([P, B * S], f32, tag="riota")
            sero_sb = pp.tile([P, B * NSTEP], f32, tag="sero")
            ident = pp.tile([P, P], f32, tag="ident")
            w1a = pp.tile([P, 128], f32, tag="w1a")
            w1b = pp.tile([P, 128], f32, tag="w1b")
            w1c = pp.tile([P, 128], f32, tag="w1c")
            w2t = pp.tile([P, 128], f32, tag="w2t")
            wh1t = pp.tile([P, 128], f32, tag="wh1t")
            wh2t = pp.tile([P, 64], f32, tag="wh2t")
            w1a6 = pp.tile([P, 128], f16, tag="w1a6")
            w1b6 = pp.tile([P, 128], f16, tag="w1b6")
            w1c6 = pp.tile([P, 128], f16, tag="w1c6")
            w2t6 = pp.tile([P, 128], f16, tag="w2t6")
            wh1t6 = pp.tile([P, 128], f16, tag="wh1t6")
            wh2t6 = pp.tile([P, 64], f16, tag="wh2t6")
            id16 = pp.tile([P, P], f16, tag="id16")
            w1f = pp.tile([P, 128], f32, tag="w1f")
            b1s = pp.tile([P, 1], f32, tag="b1s")
            b2s = pp.tile([P, 1], f32, tag="b2s")
            bh1s = pp.tile([P, 1], f32, tag="bh1s")
            t0 = pp.tile([P, 4], f32, tag="t0")

            # ---- setup ----
            import concourse.bass as bass
            make_identity(nc, ident[:])
            # offs: host-prepared [p, 4*i + b]; ohtab: [(b*128+p)*4096 + 64*i + n]
            nc.sync.dma_start(out=offs_sb[:], in_=offs_d[:])
            nc.sync.dma_start(
                out=_v(oht_sb[:], 0, [[NSTEP * S, B], [1, NSTEP * S]]),
                in_=bass.AP(oht_d[:].tensor, 0,
                            [[NSTEP * S, P], [P * NSTEP * S, B], [1, NSTEP * S]]),
            )
            nc.sync.dma_start(out=w1a[:], in_=w1_d[0:128, :])
            nc.sync.dma_start(out=w1b[:], in_=w1_d[128:256, :])
            nc.sync.dma_start(out=w1c[0:68, :], in_=w1_d[256:324, :])
            nc.sync.dma_start(out=w1f[0:4, :], in_=w1_d[324:328, :])
            nc.sync.dma_start(out=w2t[:], in_=w2_d[:])
            nc.sync.dma_start(out=wh1t[:], in_=wh1_d[:])
            nc.sync.dma_start(out=wh2t[:], in_=wh2_d[:])
            nc.sync.dma_start(out=b1s[:], in_=b1_d[:].rearrange("(d o) -> d o", o=1))
            nc.sync.dma_start(out=b2s[:], in_=b2_d[:].rearrange("(d o) -> d o", o=1))
            nc.sync.dma_start(out=bh1s[:], in_=bh1_d[:].rearrange("(d o) -> d o", o=1))

            nc.vector.memset(qos[:], -3.0)
            nc.vector.memset(_v(C[0][:], 1, [[4, B]]), 1.0)   # avail
            nc.vector.memset(_v(C[0][:], 2, [[4, B]]), 3.0)   # thr
            nc.vector.memset(_v(C[0][:], 3, [[4, B]]), 1.0)   # rel
            nc.gpsimd.iota(riota_i[:].rearrange("p (a b) -> p a b", a=B),
                           pattern=[[0, B], [-1, S]], base=S, channel_multiplier=0)
            nc.vector.tensor_copy(riota[:], riota_i[:])
            nc.vector.tensor_copy(w1a6[:], w1a[:])
            nc.vector.tensor_copy(w1b6[:], w1b[:])
            nc.vector.memset(w1c6[:], 0.0)
            nc.vector.tensor_copy(w1c6[0:68, :], w1c[0:68, :])
            nc.vector.tensor_copy(w2t6[:], w2t[:])
            nc.vector.tensor_copy(wh1t6[:], wh1t[:])
            nc.vector.tensor_copy(wh2t6[:], wh2t[:])
            nc.vector.tensor_copy(id16[:], ident[:])

            tm_flat = tm_d[:]

            def gather(i):
                k = i % NG
                for b in range(B):
                    nc.gpsimd.indirect_dma_start(
                        out=G[k][:, BW * b:BW * b + 448],
                        out_offset=None,
                        in_=tm_flat,
                        in_offset=IndirectOffsetOnAxis(
                            ap=offs_sb[:, B * i + b:B * i + b + 1], axis=0),
                    )

            for i in range(NG):
                gather(i)

            for i in range(NSTEP):
                k = i % NG
                g = G[k]
                A, Cb = C[i % 2], C[(i + 1) % 2]

                # 1) rt = max_n task64 * qos  (gpsimd mul + DVE reduce)
                prod = wp.tile([P, B * S], f32, tag="prod")
                nc.gpsimd.tensor_tensor(
                    out=prod[:], in0=_v(g[:].bitcast(f16), F_TH, [[BW16, B], [1, S]]),
                    in1=qos[:], op=AOp.mult)
                rt_dst = _v(A[:], 0, [[4, B]])
                if i == 0:
                    nc.vector.tensor_reduce(
                        out=t0[:], in_=prod[:].rearrange("p (a b) -> p a b", a=B),
                        axis=AX.X, op=AOp.max)
                    nc.vector.tensor_scalar_add(out=rt_dst, in0=t0[:], scalar1=-3.0)
                else:
                    nc.vector.tensor_reduce(
                        out=rt_dst, in_=prod[:].rearrange("p (a b) -> p a b", a=B),
                        axis=AX.X, op=AOp.max)

                # 3) transposes -> stateT chunks (PSUM), copies -> SBUF
                g16 = g[:].bitcast(f16)
                st = [wp.tile([P, 1024], f16, tag=f"st{c}", name=f"st{c}")
                      for c in range(3)]
                for b in range(B):
                    cb = BW16 * b
                    for c in range(3):
                        nc.sync.dma_start_transpose(
                            out=st[c][:, P * b:P * (b + 1)],
                            in_=bass.AP(g16.tensor, g16.offset + cb + F_TH + 128 * c,
                                        [g16.ap[0], [1, 128]]))
                        nc.sync.dma_start_transpose(
                            out=st[c][:, 512 + P * b:512 + P * (b + 1)],
                            in_=bass.AP(g16.tensor, g16.offset + cb + F_TL + 128 * c,
                                        [g16.ap[0], [1, 128]]))

                # carries transposed to [4, 512] (fp32 rank-4 feat contribution)
                prt = pqv.tile([4, 512], f32, tag="prt", name="prt")
                for b in range(B):
                    nc.tensor.transpose(out=prt[0:4, P * b:P * (b + 1)],
                                        in_=A[:, 4 * b:4 * b + 4],
                                        identity=ident[:])
                rts = wp.tile([4, 512], f32, tag="rts", name="rts")
                nc.scalar.copy(out=rts[0:4, :], in_=prt[0:4, :])

                # 4) MLP chain: 2-pass W1 (th + tl), fp16 mid/last
                ph = pml.tile([P, 512], f32, tag="ph")
                nc.tensor.matmul(ph[:], w1a6[:], st[0][:, 0:512], start=True, stop=False)
                nc.tensor.matmul(ph[:], w1b6[:], st[1][:, 0:512], start=False, stop=False)
                nc.tensor.matmul(ph[:], w1c6[:], st[2][:, 0:512], start=False, stop=False)
                nc.tensor.matmul(ph[:], w1a6[:], st[0][:, 512:1024], start=False, stop=False)
                nc.tensor.matmul(ph[:], w1b6[:], st[1][:, 512:1024], start=False, stop=False)
                nc.tensor.matmul(ph[:], w1c6[:], st[2][:, 512:1024], start=False, stop=False)
                nc.tensor.matmul(ph[:], w1f[0:4, :], rts[0:4, :], start=False, stop=True)
                hs32 = wp.tile([P, 512], f32, tag="hs32")
                nc.scalar.activation(out=hs32[:], in_=ph[:], func=AF.Silu, bias=b1s[:])
                hs = wp.tile([P, 512], f16, tag="hs")
                nc.scalar.activation(out=hs[:], in_=hs32[:], func=AF.Copy,
                                     scale=1.0 / 256.0, bias=0.0)

                pe = pml.tile([P, 512], f32, tag="pe")
                nc.tensor.matmul(pe[:], w2t6[:], hs[:], start=True, stop=True)
                xs32 = wp.tile([P, 512], f32, tag="xs32")
                nc.scalar.activation(out=xs32[:], in_=pe[:], func=AF.Silu, bias=b2s[:])
                xs = wp.tile([P, 512], f16, tag="xs")
                nc.scalar.activation(out=xs[:], in_=xs32[:], func=AF.Copy,
                                     scale=1.0 / 256.0, bias=0.0)

                ph2 = pml.tile([P, 512], f32, tag="ph2")
                nc.tensor.matmul(ph2[:], wh1t6[:], xs[:], start=True, stop=True)
                h2s32 = wp.tile([P, 512], f32, tag="h2s32")
                nc.scalar.activation(out=h2s32[:], in_=ph2[:], func=AF.Silu, bias=bh1s[:])
                h2s = wp.tile([P, 512], f16, tag="h2s")
                nc.scalar.activation(out=h2s[:], in_=h2s32[:], func=AF.Copy,
                                     scale=1.0 / 256.0, bias=0.0)

                pqvt = pqv.tile([P, B * S], f32, tag="pqv")
                for b in range(B):
                    nc.tensor.matmul(pqvt[:, S * b:S * (b + 1)],
                                     h2s[:, P * b:P * (b + 1)], wh2t6[:],
                                     start=True, stop=True)

                # 5) masked argmax (additive mask+bias already in G's M cols)
                qvm = wp.tile([P, B * S], f32, tag="qvm")
                nc.vector.tensor_tensor(out=qvm[:], in0=pqvt[:],
                                        in1=_v(g[:], C_M, [[BW, B], [1, S]]), op=AOp.add)
                mx = wp.tile([P, B], f32, tag="mx")
                nc.vector.tensor_reduce(out=mx[:],
                                        in_=qvm[:].rearrange("p (a b) -> p a b", a=B),
                                        axis=AX.X, op=AOp.max)
                oh = wp.tile([P, B * S], f32, tag="oh")
                nc.vector.tensor_tensor(out=oh[:], in0=qvm[:],
                                        in1=mx[:].to_broadcast([P, B, S]), op=AOp.is_equal)
                serv = wp.tile([P, B * S], f32, tag="serv")
                nc.vector.tensor_tensor(out=serv[:], in0=oh[:], in1=riota[:], op=AOp.mult)
                nc.vector.tensor_reduce(
                    out=_v(sero_sb[:], B * i, [[1, B]]),
                    in_=serv[:].rearrange("p (a b) -> p a b", a=B),
                    axis=AX.X, op=AOp.max)

                # 6) sq = service features at argmax: g-mul (gpsimd) + reduce (DVE)
                gm = wp.tile([P, B * S * 4], f32, tag="gm")
                nc.gpsimd.tensor_tensor(
                    out=gm[:], in0=_v(g[:].bitcast(f16), F_TH + 64, [[BW16, B], [4, S], [1, 4]]),
                    in1=_v(oh[:], 0, [[S, B], [1, S], [0, 4]]), op=AOp.mult)
                sq = wp.tile([P, 16], f32, tag="sq")
                nc.vector.tensor_reduce(
                    out=sq[:], in_=_v(gm[:], 0, [[S * 4, B], [1, 4], [4, S]]),
                    axis=AX.X, op=AOp.add)

                # 7) carry updates into Cb
                nc.vector.tensor_tensor(out=_v(Cb[:], 0, [[4, B]]),
                                        in0=_v(sq[:], 0, [[4, B]]),
                                        in1=_v(A[:], 0, [[4, B]]), op=AOp.add)
                nc.vector.tensor_tensor(out=_v(Cb[:], 1, [[4, B], [2, 2]]),
                                        in0=_v(sq[:], 1, [[4, B], [2, 2]]),
                                        in1=_v(A[:], 1, [[4, B], [2, 2]]), op=AOp.mult)
                nc.vector.tensor_tensor(out=_v(Cb[:], 2, [[4, B]]),
                                        in0=_v(sq[:], 2, [[4, B]]),
                                        in1=_v(A[:], 2, [[4, B]]), op=AOp.min)

                # 8) qos scatter: qos[q, topo] = new_rt
                nc.vector.copy_predicated(
                    out=qos[:].rearrange("p (a b) -> p a b", a=B),
                    mask=_v(oht_sb[:], S * i, [[NSTEP * S, B], [1, S]]),
                    data=_v(Cb[:], 0, [[4, B], [0, S]]))

                if i + NG < NSTEP:
                    gather(i + NG)

            nc.sync.dma_start(out=sero_d[:], in_=sero_sb[:])

    nc.compile()
    return nc


def _host_prep(tasks, constraints, masks, topologicals, bh2):
    """Build fused TM table, reversed topo, gather offsets; per-core shards."""
    Qf = tasks.shape[0]
    ncores = Qf // QL
    M = (masks.astype(np.float32) - 1.0) * 1e9 + bh2[None, None, :].astype(np.float32)
    th = tasks.astype(np.float16)                               # [Q, 64, 320]
    tl = (tasks - th.astype(np.float32)).astype(np.float16)
    cons_h = constraints.astype(np.float16)                     # [Q, 4]
    cons_l = (constraints - cons_h.astype(np.float32)).astype(np.float16)
    tm = np.zeros((Qf, 64, 448), np.float32)
    tm[:, :, 0:64] = M
    v16 = tm.view(np.float16)                                   # [Q, 64, 896]
    v16[:, :, 128:448] = th
    v16[:, :, 448:452] = cons_h[:, None, :]
    v16[:, :, 512:832] = tl
    v16[:, :, 832:836] = cons_l[:, None, :]
    topot = topologicals[:, ::-1].astype(np.int32)              # [Q, 64] reversed
    ohtab = (topot[..., None] == np.arange(64, dtype=np.int32)[None, None, :]).astype(np.int8)
    ql = np.arange(Qf, dtype=np.int32) % QL
    offs_qi = ql[:, None] * 64 + topot                          # [Q, 64]
    # per-core [p, 4*i + b] layout for contiguous per-step offset slices
    offs = offs_qi.reshape(ncores, B, P, NSTEP).transpose(0, 2, 3, 1)  # [c, p, i, b]
    offs = np.ascontiguousarray(offs.reshape(ncores, P, NSTEP * B))
    return tm, topot, offs, ohtab


def kernel(tasks, constraints, masks, topologicals,
           W1, b1, W2, b2, Wh1, bh1, Wh2, bh2):
    from concourse.bass_utils import run_bass_kernel_spmd

    tasks = np.asarray(tasks, dtype=np.float32)
    constraints = np.asarray(constraints, dtype=np.float32)
    masks = np.asarray(masks)
    topologicals = np.asarray(topologicals)
    W1 = np.asarray(W1, dtype=np.float32)
    W2 = np.asarray(W2, dtype=np.float32)
    Wh1 = np.asarray(Wh1, dtype=np.float32)
    Wh2 = np.asarray(Wh2, dtype=np.float32)
    b1 = np.asarray(b1, dtype=np.float32)
    b2 = np.asarray(b2, dtype=np.float32)
    bh1 = np.asarray(bh1, dtype=np.float32)
    bh2 = np.asarray(bh2, dtype=np.float32)

    tm, topot, offs, ohtab = _host_prep(tasks, constraints, masks, topologicals, bh2)

    if "nc" not in _cached:
        _cached["nc"] = build_program()
    nc = _cached["nc"]

    in_maps = []
    W2s = W2 * 256.0
    Wh1s = Wh1 * 256.0
    Wh2s = Wh2 * 256.0
    for c in range(NC):
        sl = slice(c * QL, (c + 1) * QL)
        in_maps.append({
            "tm": np.ascontiguousarray(tm[sl].reshape(QL * 64, 448)),
            "offs": offs[c],
            "ohtab": np.ascontiguousarray(ohtab[sl].reshape(-1)),
            "w1": W1, "w2": W2s, "wh1": Wh1s, "wh2": Wh2s,
            "b1": b1, "b2": b2, "bh1": bh1,
        })

    trace = bool(int(os.environ.get("KERNEL_TRACE", "0")))
    res = run_bass_kernel_spmd(nc, in_maps, core_ids=list(range(NC)), trace=trace)
    _cached["last_result"] = res

    ret = np.zeros((tasks.shape[0], 64), np.float32)
    rows = np.arange(tasks.shape[0])
    for c in range(NC):
        sero = res.results[c]["sero"]                 # [128, 4*64]
        ser = 64.0 - sero.reshape(P, NSTEP, B)        # [p, i, b]
        ser = ser.transpose(2, 0, 1).reshape(QL, NSTEP)  # [q_local, i]
        sl = slice(c * QL, (c + 1) * QL)
        for i in range(NSTEP):
            np.add.at(ret, (rows[sl], topot[sl, i]), ser[:, i])
    return ret.astype(np.int16)


# revision 22
# speedup vs baseline: 1.9736x; 1.9736x over previous
"""Trainium2 Bass kernel for nn_DQNDecision (64-step GNN scan).

Self-contained: hardcodes shapes. kernel(**inputs) -> [4096, 64] int16.

Strategy (see DESIGN.md): data-parallel over queries (512/core x 8 cores).
Host fuses masks+bh2 into an additive-mask table TM = [(mask-1)*1e9+bh2, task]
([Q,64,384] f32), precomputes gather offsets from topologicals. Device runs
the 64-step scan: indirect-gather node rows, PE-transpose into matmul layout,
fp32 MLP chain (weights stationary, activations as moving operand, final layer
flipped to produce query-major qv), masked argmax via reduce/is_equal, one-hot
service-feature extraction, carry updates, qos scatter via copy_predicated.
Device outputs (64 - argmax_index) per (query, step); host rebuilds ret.
"""

import os
import numpy as np

P = 128          # partitions
B = 4            # query blocks per core
QL = P * B       # queries per core
NC = 8           # cores
Q = QL * NC      # 4096
NSTEP = 64
S = 64           # services
ND = 320         # task feature width
BW = 400         # gather-tile block width (64 M + 320 task + 4 const + 4 feat + 8 pad)
GW = B * BW      # gather tile free size
NG = 5           # gather buffer depth (prefetch)
# column offsets within a block of the gather tile
C_M = 0          # additive mask+bias (64)
C_T = 64         # task (320)
C_CONST = 384    # constraints (4)
C_FEAT = 388     # rt, avail, thr, rel (4)

_cached = {}


def _v(tile_ap, off, dims):
    """Custom free-dim view of a tile AP: dims = [[step, count], ...] (elements)."""
    import concourse.bass as bass
    return bass.AP(tile_ap.tensor, tile_ap.offset + off, [tile_ap.ap[0]] + dims)


def build_program():
    import concourse.bacc as bacc
    import concourse.mybir as mybir
    from concourse.tile import TileContext
    from concourse.masks import make_identity
    from concourse.bass import IndirectOffsetOnAxis

    f32 = mybir.dt.float32
    i32 = mybir.dt.int32
    AOp = mybir.AluOpType
    AF = mybir.ActivationFunctionType
    AX = mybir.AxisListType

    nc = bacc.Bacc(
        "TRN2", target_bir_lowering=False, debug=False,
        enable_asserts=False, num_devices=NC,
    )

    # ---- DRAM IO (per-core shard) ----
    tm_d = nc.dram_tensor("tm", [QL * 64, 384], f32, kind="ExternalInput")
    offs_d = nc.dram_tensor("offs", [P, NSTEP * B], i32, kind="ExternalInput")
    topot_d = nc.dram_tensor("topot", [QL * NSTEP], i32, kind="ExternalInput")
    cons_d = nc.dram_tensor("cons", [QL * 4], f32, kind="ExternalInput")
    w1_d = nc.dram_tensor("w1", [328, 128], f32, kind="ExternalInput")
    w2_d = nc.dram_tensor("w2", [128, 128], f32, kind="ExternalInput")
    wh1_d = nc.dram_tensor("wh1", [128, 128], f32, kind="ExternalInput")
    wh2_d = nc.dram_tensor("wh2", [128, 64], f32, kind="ExternalInput")
    b1_d = nc.dram_tensor("b1", [128], f32, kind="ExternalInput")
    b2_d = nc.dram_tensor("b2", [128], f32, kind="ExternalInput")
    bh1_d = nc.dram_tensor("bh1", [128], f32, kind="ExternalInput")
    sero_d = nc.dram_tensor("sero", [P, B * NSTEP], f32, kind="ExternalOutput")

    with TileContext(nc) as tc:
        with (
            tc.tile_pool(name="pers", bufs=1) as pp,
            tc.tile_pool(name="work", bufs=2) as wp,
            tc.tile_pool(name="ps_ch", bufs=1, space="PSUM") as pch,
            tc.tile_pool(name="ps_mlp", bufs=1, space="PSUM") as pml,
            tc.tile_pool(name="ps_qv", bufs=1, space="PSUM") as pqv,
        ):
            # ---- persistent tiles ----
            G = [pp.tile([P, GW], f32, tag=f"G{k}", name=f"G{k}") for k in range(NG)]
            qos = pp.tile([P, B * S], f32, tag="qos")
            C = [pp.tile([P, 16], f32, tag=f"C{j}", name=f"C{j}") for j in range(2)]
            offs_sb = pp.tile([P, B * NSTEP], i32, tag="offs")
            topot_sb = pp.tile([P, B * NSTEP], i32, tag="topot")
            iota_sb = pp.tile([P, B * S], i32, tag="iota")
            riota_i = pp.tile([P, B * S], i32, tag="riota_i")
            riota = pp.tile([P, B * S], f32, tag="riota")
            iota_f = pp.tile([P, B * S], f32, tag="iota_f")
            topot_f = pp.tile([P, B * NSTEP], f32, tag="topot_f")
            sero_sb = pp.tile([P, B * NSTEP], f32, tag="sero")
            ident = pp.tile([P, P], f32, tag="ident")
            w1a = pp.tile([P, 128], f32, tag="w1a")
            w1b = pp.tile([P, 128], f32, tag="w1b")
            w1c = pp.tile([P, 128], f32, tag="w1c")
            w2t = pp.tile([P, 128], f32, tag="w2t")
            wh1t = pp.tile([P, 128], f32, tag="wh1t")
            wh2t = pp.tile([P, 64], f32, tag="wh2t")
            w1f = pp.tile([P, 128], f32, tag="w1f")
            b1s = pp.tile([P, 1], f32, tag="b1s")
            b2s = pp.tile([P, 1], f32, tag="b2s")
            bh1s = pp.tile([P, 1], f32, tag="bh1s")
            t0 = pp.tile([P, 4], f32, tag="t0")

            # ---- setup ----
            import concourse.bass as bass
            make_identity(nc, ident[:])
            # offs: host-prepared [p, 4*i + b]; topot: DRAM[(b*128+p)*64+i] -> SBUF[p, 64*b+i]
            nc.sync.dma_start(out=offs_sb[:], in_=offs_d[:])
            nc.sync.dma_start(
                out=_v(topot_sb[:], 0, [[NSTEP, B], [1, NSTEP]]),
                in_=bass.AP(topot_d[:].tensor, 0,
                            [[NSTEP, P], [P * NSTEP, B], [1, NSTEP]]),
            )
            nc.sync.dma_start(out=w1a[:], in_=w1_d[0:128, :])
            nc.sync.dma_start(out=w1b[:], in_=w1_d[128:256, :])
            nc.sync.dma_start(out=w1c[0:68, :], in_=w1_d[256:324, :])
            nc.sync.dma_start(out=w1f[0:4, :], in_=w1_d[324:328, :])
            nc.sync.dma_start(out=w2t[:], in_=w2_d[:])
            nc.sync.dma_start(out=wh1t[:], in_=wh1_d[:])
            nc.sync.dma_start(out=wh2t[:], in_=wh2_d[:])
            nc.sync.dma_start(out=b1s[:], in_=b1_d[:].rearrange("(d o) -> d o", o=1))
            nc.sync.dma_start(out=b2s[:], in_=b2_d[:].rearrange("(d o) -> d o", o=1))
            nc.sync.dma_start(out=bh1s[:], in_=bh1_d[:].rearrange("(d o) -> d o", o=1))
            # constraints into each gather buffer's C_CONST columns
            for k in range(NG):
                nc.sync.dma_start(
                    out=_v(G[k][:], C_CONST, [[BW, B], [1, 4]]),
                    in_=bass.AP(cons_d[:].tensor, 0, [[4, P], [P * 4, B], [1, 4]]),
                )
            nc.vector.memset(qos[:], -3.0)
            nc.vector.memset(_v(C[0][:], 1, [[4, B]]), 1.0)   # avail
            nc.vector.memset(_v(C[0][:], 2, [[4, B]]), 3.0)   # thr
            nc.vector.memset(_v(C[0][:], 3, [[4, B]]), 1.0)   # rel
            nc.gpsimd.iota(iota_sb[:].rearrange("p (a b) -> p a b", a=B),
                           pattern=[[0, B], [1, S]], base=0, channel_multiplier=0)
            nc.gpsimd.iota(riota_i[:].rearrange("p (a b) -> p a b", a=B),
                           pattern=[[0, B], [-1, S]], base=S, channel_multiplier=0)
            nc.vector.tensor_copy(riota[:], riota_i[:])
            nc.vector.tensor_copy(iota_f[:], iota_sb[:])
            nc.vector.tensor_copy(topot_f[:], topot_sb[:])

            tm_flat = tm_d[:]

            def gather(i):
                k = i % NG
                for b in range(B):
                    nc.gpsimd.indirect_dma_start(
                        out=G[k][:, BW * b:BW * b + 384],
                        out_offset=None,
                        in_=tm_flat,
                        in_offset=IndirectOffsetOnAxis(
                            ap=offs_sb[:, B * i + b:B * i + b + 1], axis=0),
                    )

            for i in range(NG):
                gather(i)

            for i in range(NSTEP):
                k = i % NG
                g = G[k]
                A, Cb = C[i % 2], C[(i + 1) % 2]

                # 1) rt = max_n task64 * qos  (gpsimd mul + DVE reduce)
                prod = wp.tile([P, B * S], f32, tag="prod")
                nc.gpsimd.tensor_tensor(
                    out=prod[:], in0=_v(g[:], C_T, [[BW, B], [1, S]]),
                    in1=qos[:], op=AOp.mult)
                rt_dst = _v(A[:], 0, [[4, B]])
                if i == 0:
                    nc.vector.tensor_reduce(
                        out=t0[:], in_=prod[:].rearrange("p (a b) -> p a b", a=B),
                        axis=AX.X, op=AOp.max)
                    nc.vector.tensor_scalar_add(out=rt_dst, in0=t0[:], scalar1=-3.0)
                else:
                    nc.vector.tensor_reduce(
                        out=rt_dst, in_=prod[:].rearrange("p (a b) -> p a b", a=B),
                        axis=AX.X, op=AOp.max)

                # 3) transposes -> stateT chunks (PSUM), copies -> SBUF
                pc0 = pch.tile([P, 512], f32, tag="pc0")
                pc1 = pch.tile([P, 512], f32, tag="pc1")
                pc2 = pch.tile([P, 512], f32, tag="pc2")
                for b in range(B):
                    cb = BW * b
                    nc.tensor.transpose(out=pc0[:, P * b:P * (b + 1)],
                                        in_=g[:, cb + C_T:cb + C_T + 128], identity=ident[:])
                    nc.tensor.transpose(out=pc1[:, P * b:P * (b + 1)],
                                        in_=g[:, cb + C_T + 128:cb + C_T + 256], identity=ident[:])
                    nc.tensor.transpose(out=pc2[0:68, P * b:P * (b + 1)],
                                        in_=g[:, cb + 320:cb + 388], identity=ident[:])
                st0 = wp.tile([P, 512], f32, tag="st0")
                st1 = wp.tile([P, 512], f32, tag="st1")
                st2 = wp.tile([P, 512], f32, tag="st2")
                nc.scalar.copy(out=st0[:], in_=pc0[:])
                nc.scalar.copy(out=st1[:], in_=pc1[:])
                nc.vector.tensor_copy(out=st2[0:68, :], in_=pc2[0:68, :])

                # 4) MLP chain (weights stationary, fp32)
                ph = pml.tile([P, 512], f32, tag="ph")
                nc.tensor.matmul(ph[:], w1a[:], st0[:], start=True, stop=False)
                nc.tensor.matmul(ph[:], w1b[:], st1[:], start=False, stop=False)
                nc.tensor.matmul(ph[:], w1c[0:68, :], st2[0:68, :], start=False, stop=False)
                prt = pqv.tile([4, 512], f32, tag="prt", name="prt")
                for b in range(B):
                    nc.tensor.transpose(out=prt[0:4, P * b:P * (b + 1)],
                                        in_=A[:, 4 * b:4 * b + 4],
                                        identity=ident[:])
                rts = wp.tile([4, 512], f32, tag="rts", name="rts")
                nc.scalar.copy(out=rts[0:4, :], in_=prt[0:4, :])
                nc.tensor.matmul(ph[:], w1f[0:4, :], rts[0:4, :], start=False, stop=True)
                hs = wp.tile([P, 512], f32, tag="hs")
                nc.scalar.activation(out=hs[:], in_=ph[:], func=AF.Silu, bias=b1s[:])

                pe = pml.tile([P, 512], f32, tag="pe")
                nc.tensor.matmul(pe[:], w2t[:], hs[:], start=True, stop=True)
                xs = wp.tile([P, 512], f32, tag="xs")
                nc.scalar.activation(out=xs[:], in_=pe[:], func=AF.Silu, bias=b2s[:])

                ph2 = pml.tile([P, 512], f32, tag="ph2")
                nc.tensor.matmul(ph2[:], wh1t[:], xs[:], start=True, stop=True)
                h2s = wp.tile([P, 512], f32, tag="h2s")
                nc.scalar.activation(out=h2s[:], in_=ph2[:], func=AF.Silu, bias=bh1s[:])

                pqvt = pqv.tile([P, B * S], f32, tag="pqv")
                for b in range(B):
                    nc.tensor.matmul(pqvt[:, S * b:S * (b + 1)],
                                     h2s[:, P * b:P * (b + 1)], wh2t[:],
                                     start=True, stop=True)

                # 5) masked argmax (additive mask+bias already in G's M cols)
                qvm = wp.tile([P, B * S], f32, tag="qvm")
                nc.vector.tensor_tensor(out=qvm[:], in0=pqvt[:],
                                        in1=_v(g[:], C_M, [[BW, B], [1, S]]), op=AOp.add)
                mx = wp.tile([P, B], f32, tag="mx")
                nc.vector.tensor_reduce(out=mx[:],
                                        in_=qvm[:].rearrange("p (a b) -> p a b", a=B),
                                        axis=AX.X, op=AOp.max)
                oh = wp.tile([P, B * S], f32, tag="oh")
                nc.vector.tensor_tensor(out=oh[:], in0=qvm[:],
                                        in1=mx[:].to_broadcast([P, B, S]), op=AOp.is_equal)
                serv = wp.tile([P, B * S], f32, tag="serv")
                nc.vector.tensor_tensor(out=serv[:], in0=oh[:], in1=riota[:], op=AOp.mult)
                nc.vector.tensor_reduce(
                    out=_v(sero_sb[:], B * i, [[1, B]]),
                    in_=serv[:].rearrange("p (a b) -> p a b", a=B),
                    axis=AX.X, op=AOp.max)

                # 6) sq = service features at argmax: g-mul (gpsimd) + reduce (DVE)
                gm = wp.tile([P, B * S * 4], f32, tag="gm")
                nc.gpsimd.tensor_tensor(
                    out=gm[:], in0=_v(g[:], C_T + 64, [[BW, B], [4, S], [1, 4]]),
                    in1=_v(oh[:], 0, [[S, B], [1, S], [0, 4]]), op=AOp.mult)
                sq = wp.tile([P, 16], f32, tag="sq")
                nc.vector.tensor_reduce(
                    out=sq[:], in_=_v(gm[:], 0, [[S * 4, B], [1, 4], [4, S]]),
                    axis=AX.X, op=AOp.add)

                # 7) carry updates into Cb
                nc.vector.tensor_tensor(out=_v(Cb[:], 0, [[4, B]]),
                                        in0=_v(sq[:], 0, [[4, B]]),
                                        in1=_v(A[:], 0, [[4, B]]), op=AOp.add)
                nc.vector.tensor_tensor(out=_v(Cb[:], 1, [[4, B], [2, 2]]),
                                        in0=_v(sq[:], 1, [[4, B], [2, 2]]),
                                        in1=_v(A[:], 1, [[4, B], [2, 2]]), op=AOp.mult)
                nc.vector.tensor_tensor(out=_v(Cb[:], 2, [[4, B]]),
                                        in0=_v(sq[:], 2, [[4, B]]),
                                        in1=_v(A[:], 2, [[4, B]]), op=AOp.min)

                # 8) qos scatter: qos[q, topo] = new_rt
                oht = wp.tile([P, B * S], i32, tag="oht")
                nc.vector.tensor_tensor(
                    out=oht[:], in0=iota_sb[:],
                    in1=_v(topot_sb[:], i, [[NSTEP, B], [0, S]]), op=AOp.is_equal)
                nc.vector.copy_predicated(
                    out=qos[:].rearrange("p (a b) -> p a b", a=B),
                    mask=oht[:].rearrange("p (a b) -> p a b", a=B),
                    data=_v(Cb[:], 0, [[4, B], [0, S]]))

                if i + NG < NSTEP:
                    gather(i + NG)

            nc.sync.dma_start(out=sero_d[:], in_=sero_sb[:])

    nc.compile()
    return nc


def _host_prep(tasks, constraints, masks, topologicals, bh2):
    """Build fused TM table, reversed topo, gather offsets; per-core shards."""
    Qf = tasks.shape[0]
    ncores = Qf // QL
    M = (masks.astype(np.float32) - 1.0) * 1e9 + bh2[None, None, :].astype(np.float32)
    tm = np.concatenate([M, tasks], axis=2)                     # [Q, 64, 384]
    topot = topologicals[:, ::-1].astype(np.int32)              # [Q, 64] reversed
    ql = np.arange(Qf, dtype=np.int32) % QL
    offs_qi = ql[:, None] * 64 + topot                          # [Q, 64]
    # per-core [p, 4*i + b] layout for contiguous per-step offset slices
    offs = offs_qi.reshape(ncores, B, P, NSTEP).transpose(0, 2, 3, 1)  # [c, p, i, b]
    offs = np.ascontiguousarray(offs.reshape(ncores, P, NSTEP * B))
    return tm, topot, offs


def kernel(tasks, constraints, masks, topologicals,
           W1, b1, W2, b2, Wh1, bh1, Wh2, bh2):
    from concourse.bass_utils import run_bass_kernel_spmd

    tasks = np.asarray(tasks, dtype=np.float32)
    constraints = np.asarray(constraints, dtype=np.float32)
    masks = np.asarray(masks)
    topologicals = np.asarray(topologicals)
    W1 = np.asarray(W1, dtype=np.float32)
    W2 = np.asarray(W2, dtype=np.float32)
    Wh1 = np.asarray(Wh1, dtype=np.float32)
    Wh2 = np.asarray(Wh2, dtype=np.float32)
    b1 = np.asarray(b1, dtype=np.float32)
    b2 = np.asarray(b2, dtype=np.float32)
    bh1 = np.asarray(bh1, dtype=np.float32)
    bh2 = np.asarray(bh2, dtype=np.float32)

    tm, topot, offs = _host_prep(tasks, constraints, masks, topologicals, bh2)

    if "nc" not in _cached:
        _cached["nc"] = build_program()
    nc = _cached["nc"]

    in_maps = []
    for c in range(NC):
        sl = slice(c * QL, (c + 1) * QL)
        in_maps.append({
            "tm": np.ascontiguousarray(tm[sl].reshape(QL * 64, 384)),
            "offs": offs[c],
            "topot": np.ascontiguousarray(topot[sl].reshape(-1)),
            "cons": np.ascontiguousarray(constraints[sl].reshape(-1)),
            "w1": W1, "w2": W2, "wh1": Wh1, "wh2": Wh2,
            "b1": b1, "b2": b2, "bh1": bh1,
        })

    trace = bool(int(os.environ.get("KERNEL_TRACE", "0")))
    res = run_bass_kernel_spmd(nc, in_maps, core_ids=list(range(NC)), trace=trace)
    _cached["last_result"] = res

    ret = np.zeros((tasks.shape[0], 64), np.float32)
    rows = np.arange(tasks.shape[0])
    for c in range(NC):
        sero = res.results[c]["sero"]                 # [128, 4*64]
        ser = 64.0 - sero.reshape(P, NSTEP, B)        # [p, i, b]
        ser = ser.transpose(2, 0, 1).reshape(QL, NSTEP)  # [q_local, i]
        sl = slice(c * QL, (c + 1) * QL)
        for i in range(NSTEP):
            np.add.at(ret, (rows[sl], topot[sl, i]), ser[:, i])
    return ret.astype(np.int16)


# revision 23
# speedup vs baseline: 1.9950x; 1.0108x over previous
"""Trainium2 Bass kernel for nn_DQNDecision (64-step GNN scan).

Self-contained: hardcodes shapes. kernel(**inputs) -> [4096, 64] int16.

Strategy (see DESIGN.md): data-parallel over queries (512/core x 8 cores).
Host fuses masks+bh2 into an additive-mask table TM = [(mask-1)*1e9+bh2, task]
([Q,64,384] f32), precomputes gather offsets from topologicals. Device runs
the 64-step scan: indirect-gather node rows, PE-transpose into matmul layout,
fp32 MLP chain (weights stationary, activations as moving operand, final layer
flipped to produce query-major qv), masked argmax via reduce/is_equal, one-hot
service-feature extraction, carry updates, qos scatter via copy_predicated.
Device outputs (64 - argmax_index) per (query, step); host rebuilds ret.
"""

import os
import numpy as np

P = 128          # partitions
B = 4            # query blocks per core
QL = P * B       # queries per core
NC = 8           # cores
Q = QL * NC      # 4096
NSTEP = 64
S = 64           # services
ND = 320         # task feature width
BW = 400         # gather-tile block width (64 M + 320 task + 4 const + 4 feat + 8 pad)
GW = B * BW      # gather tile free size
NG = 5           # gather buffer depth (prefetch)
# column offsets within a block of the gather tile
C_M = 0          # additive mask+bias (64)
C_T = 64         # task (320)
C_CONST = 384    # constraints (4)
C_FEAT = 388     # rt, avail, thr, rel (4)

_cached = {}


def _v(tile_ap, off, dims):
    """Custom free-dim view of a tile AP: dims = [[step, count], ...] (elements)."""
    import concourse.bass as bass
    return bass.AP(tile_ap.tensor, tile_ap.offset + off, [tile_ap.ap[0]] + dims)


def build_program():
    import concourse.bacc as bacc
    import concourse.mybir as mybir
    from concourse.tile import TileContext
    from concourse.masks import make_identity
    from concourse.bass import IndirectOffsetOnAxis

    f32 = mybir.dt.float32
    i32 = mybir.dt.int32
    AOp = mybir.AluOpType
    AF = mybir.ActivationFunctionType
    AX = mybir.AxisListType

    nc = bacc.Bacc(
        "TRN2", target_bir_lowering=False, debug=False,
        enable_asserts=False, num_devices=NC,
    )

    # ---- DRAM IO (per-core shard) ----
    tm_d = nc.dram_tensor("tm", [QL * 64, 384], f32, kind="ExternalInput")
    offs_d = nc.dram_tensor("offs", [P, NSTEP * B], i32, kind="ExternalInput")
    topot_d = nc.dram_tensor("topot", [QL * NSTEP], i32, kind="ExternalInput")
    cons_d = nc.dram_tensor("cons", [QL * 4], f32, kind="ExternalInput")
    w1_d = nc.dram_tensor("w1", [328, 128], f32, kind="ExternalInput")
    w2_d = nc.dram_tensor("w2", [128, 128], f32, kind="ExternalInput")
    wh1_d = nc.dram_tensor("wh1", [128, 128], f32, kind="ExternalInput")
    wh2_d = nc.dram_tensor("wh2", [128, 64], f32, kind="ExternalInput")
    b1_d = nc.dram_tensor("b1", [128], f32, kind="ExternalInput")
    b2_d = nc.dram_tensor("b2", [128], f32, kind="ExternalInput")
    bh1_d = nc.dram_tensor("bh1", [128], f32, kind="ExternalInput")
    sero_d = nc.dram_tensor("sero", [P, B * NSTEP], f32, kind="ExternalOutput")

    with TileContext(nc) as tc:
        with (
            tc.tile_pool(name="pers", bufs=1) as pp,
            tc.tile_pool(name="work", bufs=2) as wp,
            tc.tile_pool(name="ps_ch", bufs=1, space="PSUM") as pch,
            tc.tile_pool(name="ps_mlp", bufs=1, space="PSUM") as pml,
            tc.tile_pool(name="ps_qv", bufs=1, space="PSUM") as pqv,
        ):
            # ---- persistent tiles ----
            G = [pp.tile([P, GW], f32, tag=f"G{k}", name=f"G{k}") for k in range(NG)]
            qos = pp.tile([P, B * S], f32, tag="qos")
            C = [pp.tile([P, 16], f32, tag=f"C{j}", name=f"C{j}") for j in range(2)]
            offs_sb = pp.tile([P, B * NSTEP], i32, tag="offs")
            topot_sb = pp.tile([P, B * NSTEP], i32, tag="topot")
            iota_sb = pp.tile([P, B * S], i32, tag="iota")
            riota_i = pp.tile([P, B * S], i32, tag="riota_i")
            riota = pp.tile([P, B * S], f32, tag="riota")
            iota_f = pp.tile([P, B * S], f32, tag="iota_f")
            topot_f = pp.tile([P, B * NSTEP], f32, tag="topot_f")
            sero_sb = pp.tile([P, B * NSTEP], f32, tag="sero")
            ident = pp.tile([P, P], f32, tag="ident")
            w1a = pp.tile([P, 128], f32, tag="w1a")
            w1b = pp.tile([P, 128], f32, tag="w1b")
            w1c = pp.tile([P, 128], f32, tag="w1c")
            w2t = pp.tile([P, 128], f32, tag="w2t")
            wh1t = pp.tile([P, 128], f32, tag="wh1t")
            wh2t = pp.tile([P, 64], f32, tag="wh2t")
            w1f = pp.tile([P, 128], f32, tag="w1f")
            b1s = pp.tile([P, 1], f32, tag="b1s")
            b2s = pp.tile([P, 1], f32, tag="b2s")
            bh1s = pp.tile([P, 1], f32, tag="bh1s")
            t0 = pp.tile([P, 4], f32, tag="t0")

            # ---- setup ----
            import concourse.bass as bass
            make_identity(nc, ident[:])
            # offs: host-prepared [p, 4*i + b]; topot: DRAM[(b*128+p)*64+i] -> SBUF[p, 64*b+i]
            nc.sync.dma_start(out=offs_sb[:], in_=offs_d[:])
            nc.sync.dma_start(
                out=_v(topot_sb[:], 0, [[NSTEP, B], [1, NSTEP]]),
                in_=bass.AP(topot_d[:].tensor, 0,
                            [[NSTEP, P], [P * NSTEP, B], [1, NSTEP]]),
            )
            nc.sync.dma_start(out=w1a[:], in_=w1_d[0:128, :])
            nc.sync.dma_start(out=w1b[:], in_=w1_d[128:256, :])
            nc.sync.dma_start(out=w1c[0:68, :], in_=w1_d[256:324, :])
            nc.sync.dma_start(out=w1f[0:4, :], in_=w1_d[324:328, :])
            nc.sync.dma_start(out=w2t[:], in_=w2_d[:])
            nc.sync.dma_start(out=wh1t[:], in_=wh1_d[:])
            nc.sync.dma_start(out=wh2t[:], in_=wh2_d[:])
            nc.sync.dma_start(out=b1s[:], in_=b1_d[:].rearrange("(d o) -> d o", o=1))
            nc.sync.dma_start(out=b2s[:], in_=b2_d[:].rearrange("(d o) -> d o", o=1))
            nc.sync.dma_start(out=bh1s[:], in_=bh1_d[:].rearrange("(d o) -> d o", o=1))
            # constraints into each gather buffer's C_CONST columns
            for k in range(NG):
                nc.sync.dma_start(
                    out=_v(G[k][:], C_CONST, [[BW, B], [1, 4]]),
                    in_=bass.AP(cons_d[:].tensor, 0, [[4, P], [P * 4, B], [1, 4]]),
                )
            nc.vector.memset(qos[:], -3.0)
            nc.vector.memset(_v(C[0][:], 1, [[4, B]]), 1.0)   # avail
            nc.vector.memset(_v(C[0][:], 2, [[4, B]]), 3.0)   # thr
            nc.vector.memset(_v(C[0][:], 3, [[4, B]]), 1.0)   # rel
            nc.gpsimd.iota(iota_sb[:].rearrange("p (a b) -> p a b", a=B),
                           pattern=[[0, B], [1, S]], base=0, channel_multiplier=0)
            nc.gpsimd.iota(riota_i[:].rearrange("p (a b) -> p a b", a=B),
                           pattern=[[0, B], [-1, S]], base=S, channel_multiplier=0)
            nc.vector.tensor_copy(riota[:], riota_i[:])
            nc.vector.tensor_copy(iota_f[:], iota_sb[:])
            nc.vector.tensor_copy(topot_f[:], topot_sb[:])

            tm_flat = tm_d[:]

            def gather(i):
                k = i % NG
                for b in range(B):
                    nc.gpsimd.indirect_dma_start(
                        out=G[k][:, BW * b:BW * b + 384],
                        out_offset=None,
                        in_=tm_flat,
                        in_offset=IndirectOffsetOnAxis(
                            ap=offs_sb[:, B * i + b:B * i + b + 1], axis=0),
                    )

            for i in range(NG):
                gather(i)

            for i in range(NSTEP):
                k = i % NG
                g = G[k]
                A, Cb = C[i % 2], C[(i + 1) % 2]

                # 1) rt = max_n task64 * qos  (gpsimd mul + DVE reduce)
                prod = wp.tile([P, B * S], f32, tag="prod")
                nc.gpsimd.tensor_tensor(
                    out=prod[:], in0=_v(g[:], C_T, [[BW, B], [1, S]]),
                    in1=qos[:], op=AOp.mult)
                rt_dst = _v(A[:], 0, [[4, B]])
                if i == 0:
                    nc.vector.tensor_reduce(
                        out=t0[:], in_=prod[:].rearrange("p (a b) -> p a b", a=B),
                        axis=AX.X, op=AOp.max)
                    nc.vector.tensor_scalar_add(out=rt_dst, in0=t0[:], scalar1=-3.0)
                else:
                    nc.vector.tensor_reduce(
                        out=rt_dst, in_=prod[:].rearrange("p (a b) -> p a b", a=B),
                        axis=AX.X, op=AOp.max)

                # 3) transposes -> stateT chunks (PSUM), copies -> SBUF
                pc0 = pch.tile([P, 512], f32, tag="pc0")
                pc1 = pch.tile([P, 512], f32, tag="pc1")
                pc2 = pch.tile([P, 512], f32, tag="pc2")
                for b in range(B):
                    cb = BW * b
                    nc.tensor.transpose(out=pc0[:, P * b:P * (b + 1)],
                                        in_=g[:, cb + C_T:cb + C_T + 128], identity=ident[:])
                    nc.tensor.transpose(out=pc1[:, P * b:P * (b + 1)],
                                        in_=g[:, cb + C_T + 128:cb + C_T + 256], identity=ident[:])
                    nc.tensor.transpose(out=pc2[0:68, P * b:P * (b + 1)],
                                        in_=g[:, cb + 320:cb + 388], identity=ident[:])
                st0 = wp.tile([P, 512], f32, tag="st0", bufs=3)
                st1 = wp.tile([P, 512], f32, tag="st1", bufs=3)
                st2 = wp.tile([P, 512], f32, tag="st2", bufs=3)
                nc.scalar.copy(out=st0[:], in_=pc0[:])
                nc.scalar.copy(out=st1[:], in_=pc1[:])
                nc.vector.tensor_copy(out=st2[0:68, :], in_=pc2[0:68, :])

                # 4) MLP chain (weights stationary, fp32)
                ph = pml.tile([P, 512], f32, tag="ph", bufs=2)
                nc.tensor.matmul(ph[:], w1a[:], st0[:], start=True, stop=False)
                nc.tensor.matmul(ph[:], w1b[:], st1[:], start=False, stop=False)
                nc.tensor.matmul(ph[:], w1c[0:68, :], st2[0:68, :], start=False, stop=False)
                prt = pqv.tile([4, 512], f32, tag="prt", name="prt")
                for b in range(B):
                    nc.tensor.transpose(out=prt[0:4, P * b:P * (b + 1)],
                                        in_=A[:, 4 * b:4 * b + 4],
                                        identity=ident[:])
                rts = wp.tile([4, 512], f32, tag="rts", name="rts")
                nc.scalar.copy(out=rts[0:4, :], in_=prt[0:4, :])
                nc.tensor.matmul(ph[:], w1f[0:4, :], rts[0:4, :], start=False, stop=True)
                hs = wp.tile([P, 512], f32, tag="hs")
                nc.scalar.activation(out=hs[:], in_=ph[:], func=AF.Silu, bias=b1s[:])

                pe = pml.tile([P, 512], f32, tag="pe")
                nc.tensor.matmul(pe[:], w2t[:], hs[:], start=True, stop=True)
                xs = wp.tile([P, 512], f32, tag="xs")
                nc.scalar.activation(out=xs[:], in_=pe[:], func=AF.Silu, bias=b2s[:])

                ph2 = pml.tile([P, 512], f32, tag="pe")
                nc.tensor.matmul(ph2[:], wh1t[:], xs[:], start=True, stop=True)
                h2s = wp.tile([P, 512], f32, tag="h2s")
                nc.scalar.activation(out=h2s[:], in_=ph2[:], func=AF.Silu, bias=bh1s[:])

                pqvt = pqv.tile([P, B * S], f32, tag="pqv")
                for b in range(B):
                    nc.tensor.matmul(pqvt[:, S * b:S * (b + 1)],
                                     h2s[:, P * b:P * (b + 1)], wh2t[:],
                                     start=True, stop=True)

                # 5) masked argmax (additive mask+bias already in G's M cols)
                qvm = wp.tile([P, B * S], f32, tag="qvm")
                nc.vector.tensor_tensor(out=qvm[:], in0=pqvt[:],
                                        in1=_v(g[:], C_M, [[BW, B], [1, S]]), op=AOp.add)
                mx = wp.tile([P, B], f32, tag="mx")
                nc.vector.tensor_reduce(out=mx[:],
                                        in_=qvm[:].rearrange("p (a b) -> p a b", a=B),
                                        axis=AX.X, op=AOp.max)
                oh = wp.tile([P, B * S], f32, tag="oh")
                nc.vector.tensor_tensor(out=oh[:], in0=qvm[:],
                                        in1=mx[:].to_broadcast([P, B, S]), op=AOp.is_equal)
                serv = wp.tile([P, B * S], f32, tag="serv")
                nc.vector.tensor_tensor(out=serv[:], in0=oh[:], in1=riota[:], op=AOp.mult)
                nc.vector.tensor_reduce(
                    out=_v(sero_sb[:], B * i, [[1, B]]),
                    in_=serv[:].rearrange("p (a b) -> p a b", a=B),
                    axis=AX.X, op=AOp.max)

                # 6) sq = service features at argmax: g-mul (gpsimd) + reduce (DVE)
                gm = wp.tile([P, B * S * 4], f32, tag="gm")
                nc.gpsimd.tensor_tensor(
                    out=gm[:], in0=_v(g[:], C_T + 64, [[BW, B], [4, S], [1, 4]]),
                    in1=_v(oh[:], 0, [[S, B], [1, S], [0, 4]]), op=AOp.mult)
                sq = wp.tile([P, 16], f32, tag="sq")
                nc.vector.tensor_reduce(
                    out=sq[:], in_=_v(gm[:], 0, [[S * 4, B], [1, 4], [4, S]]),
                    axis=AX.X, op=AOp.add)

                # 7) carry updates into Cb
                nc.vector.tensor_tensor(out=_v(Cb[:], 0, [[4, B]]),
                                        in0=_v(sq[:], 0, [[4, B]]),
                                        in1=_v(A[:], 0, [[4, B]]), op=AOp.add)
                nc.vector.tensor_tensor(out=_v(Cb[:], 1, [[4, B], [2, 2]]),
                                        in0=_v(sq[:], 1, [[4, B], [2, 2]]),
                                        in1=_v(A[:], 1, [[4, B], [2, 2]]), op=AOp.mult)
                nc.vector.tensor_tensor(out=_v(Cb[:], 2, [[4, B]]),
                                        in0=_v(sq[:], 2, [[4, B]]),
                                        in1=_v(A[:], 2, [[4, B]]), op=AOp.min)

                # 8) qos scatter: qos[q, topo] = new_rt
                oht = wp.tile([P, B * S], i32, tag="oht")
                nc.vector.tensor_tensor(
                    out=oht[:], in0=iota_sb[:],
                    in1=_v(topot_sb[:], i, [[NSTEP, B], [0, S]]), op=AOp.is_equal)
                nc.vector.copy_predicated(
                    out=qos[:].rearrange("p (a b) -> p a b", a=B),
                    mask=oht[:].rearrange("p (a b) -> p a b", a=B),
                    data=_v(Cb[:], 0, [[4, B], [0, S]]))

                if i + NG < NSTEP:
                    gather(i + NG)

            nc.sync.dma_start(out=sero_d[:], in_=sero_sb[:])

    nc.compile()
    return nc


def _host_prep(tasks, constraints, masks, topologicals, bh2):
    """Build fused TM table, reversed topo, gather offsets; per-core shards."""
    Qf = tasks.shape[0]
    ncores = Qf // QL
    M = (masks.astype(np.float32) - 1.0) * 1e9 + bh2[None, None, :].astype(np.float32)
    tm = np.concatenate([M, tasks], axis=2)                     # [Q, 64, 384]
    topot = topologicals[:, ::-1].astype(np.int32)              # [Q, 64] reversed
    ql = np.arange(Qf, dtype=np.int32) % QL
    offs_qi = ql[:, None] * 64 + topot                          # [Q, 64]
    # per-core [p, 4*i + b] layout for contiguous per-step offset slices
    offs = offs_qi.reshape(ncores, B, P, NSTEP).transpose(0, 2, 3, 1)  # [c, p, i, b]
    offs = np.ascontiguousarray(offs.reshape(ncores, P, NSTEP * B))
    return tm, topot, offs


def kernel(tasks, constraints, masks, topologicals,
           W1, b1, W2, b2, Wh1, bh1, Wh2, bh2):
    from concourse.bass_utils import run_bass_kernel_spmd

    tasks = np.asarray(tasks, dtype=np.float32)
    constraints = np.asarray(constraints, dtype=np.float32)
    masks = np.asarray(masks)
    topologicals = np.asarray(topologicals)
    W1 = np.asarray(W1, dtype=np.float32)
    W2 = np.asarray(W2, dtype=np.float32)
    Wh1 = np.asarray(Wh1, dtype=np.float32)
    Wh2 = np.asarray(Wh2, dtype=np.float32)
    b1 = np.asarray(b1, dtype=np.float32)
    b2 = np.asarray(b2, dtype=np.float32)
    bh1 = np.asarray(bh1, dtype=np.float32)
    bh2 = np.asarray(bh2, dtype=np.float32)

    tm, topot, offs = _host_prep(tasks, constraints, masks, topologicals, bh2)

    if "nc" not in _cached:
        _cached["nc"] = build_program()
    nc = _cached["nc"]

    in_maps = []
    for c in range(NC):
        sl = slice(c * QL, (c + 1) * QL)
        in_maps.append({
            "tm": np.ascontiguousarray(tm[sl].reshape(QL * 64, 384)),
            "offs": offs[c],
            "topot": np.ascontiguousarray(topot[sl].reshape(-1)),
            "cons": np.ascontiguousarray(constraints[sl].reshape(-1)),
            "w1": W1, "w2": W2, "wh1": Wh1, "wh2": Wh2,
            "b1": b1, "b2": b2, "bh1": bh1,
        })

    trace = bool(int(os.environ.get("KERNEL_TRACE", "0")))
    res = run_bass_kernel_spmd(nc, in_maps, core_ids=list(range(NC)), trace=trace)
    _cached["last_result"] = res

    ret = np.zeros((tasks.shape[0], 64), np.float32)
    rows = np.arange(tasks.shape[0])
    for c in range(NC):
        sero = res.results[c]["sero"]                 # [128, 4*64]
        ser = 64.0 - sero.reshape(P, NSTEP, B)        # [p, i, b]
        ser = ser.transpose(2, 0, 1).reshape(QL, NSTEP)  # [q_local, i]
        sl = slice(c * QL, (c + 1) * QL)
        for i in range(NSTEP):
            np.add.at(ret, (rows[sl], topot[sl, i]), ser[:, i])
    return ret.astype(np.int16)


# revision 25
# speedup vs baseline: 1.9986x; 1.0018x over previous
"""Trainium2 Bass kernel for nn_DQNDecision (64-step GNN scan).

Self-contained: hardcodes shapes. kernel(**inputs) -> [4096, 64] int16.

Strategy (see DESIGN.md): data-parallel over queries (512/core x 8 cores).
Host fuses masks+bh2 into an additive-mask table TM = [(mask-1)*1e9+bh2, task]
([Q,64,384] f32), precomputes gather offsets from topologicals. Device runs
the 64-step scan: indirect-gather node rows, PE-transpose into matmul layout,
fp32 MLP chain (weights stationary, activations as moving operand, final layer
flipped to produce query-major qv), masked argmax via reduce/is_equal, one-hot
service-feature extraction, carry updates, qos scatter via copy_predicated.
Device outputs (64 - argmax_index) per (query, step); host rebuilds ret.
"""

import os
import numpy as np

P = 128          # partitions
B = 4            # query blocks per core
QL = P * B       # queries per core
NC = 8           # cores
Q = QL * NC      # 4096
NSTEP = 64
S = 64           # services
ND = 320         # task feature width
BW = 400         # gather-tile block width (64 M + 320 task + 4 const + 4 feat + 8 pad)
GW = B * BW      # gather tile free size
NG = 5           # gather buffer depth (prefetch)
# column offsets within a block of the gather tile
C_M = 0          # additive mask+bias (64)
C_T = 64         # task (320)
C_CONST = 384    # constraints (4)
C_FEAT = 388     # rt, avail, thr, rel (4)

_cached = {}


def _v(tile_ap, off, dims):
    """Custom free-dim view of a tile AP: dims = [[step, count], ...] (elements)."""
    import concourse.bass as bass
    return bass.AP(tile_ap.tensor, tile_ap.offset + off, [tile_ap.ap[0]] + dims)


def build_program():
    import concourse.bacc as bacc
    import concourse.mybir as mybir
    from concourse.tile import TileContext
    from concourse.masks import make_identity
    from concourse.bass import IndirectOffsetOnAxis

    f32 = mybir.dt.float32
    i32 = mybir.dt.int32
    AOp = mybir.AluOpType
    AF = mybir.ActivationFunctionType
    AX = mybir.AxisListType

    nc = bacc.Bacc(
        "TRN2", target_bir_lowering=False, debug=False,
        enable_asserts=False, num_devices=NC,
    )

    # ---- DRAM IO (per-core shard) ----
    tm_d = nc.dram_tensor("tm", [QL * 64, 384], f32, kind="ExternalInput")
    offs_d = nc.dram_tensor("offs", [P, NSTEP * B], i32, kind="ExternalInput")
    topot_d = nc.dram_tensor("topot", [QL * NSTEP], i32, kind="ExternalInput")
    cons_d = nc.dram_tensor("cons", [QL * 4], f32, kind="ExternalInput")
    w1_d = nc.dram_tensor("w1", [328, 128], f32, kind="ExternalInput")
    w2_d = nc.dram_tensor("w2", [128, 128], f32, kind="ExternalInput")
    wh1_d = nc.dram_tensor("wh1", [128, 128], f32, kind="ExternalInput")
    wh2_d = nc.dram_tensor("wh2", [128, 64], f32, kind="ExternalInput")
    b1_d = nc.dram_tensor("b1", [128], f32, kind="ExternalInput")
    b2_d = nc.dram_tensor("b2", [128], f32, kind="ExternalInput")
    bh1_d = nc.dram_tensor("bh1", [128], f32, kind="ExternalInput")
    sero_d = nc.dram_tensor("sero", [P, B * NSTEP], f32, kind="ExternalOutput")

    with TileContext(nc) as tc:
        with (
            tc.tile_pool(name="pers", bufs=1) as pp,
            tc.tile_pool(name="work", bufs=2) as wp,
            tc.tile_pool(name="ps_ch", bufs=1, space="PSUM") as pch,
            tc.tile_pool(name="ps_mlp", bufs=1, space="PSUM") as pml,
            tc.tile_pool(name="ps_qv", bufs=1, space="PSUM") as pqv,
        ):
            # ---- persistent tiles ----
            G = [pp.tile([P, GW], f32, tag=f"G{k}", name=f"G{k}") for k in range(NG)]
            qos = pp.tile([P, B * S], f32, tag="qos")
            C = [pp.tile([P, 16], f32, tag=f"C{j}", name=f"C{j}") for j in range(2)]
            offs_sb = pp.tile([P, B * NSTEP], i32, tag="offs")
            topot_sb = pp.tile([P, B * NSTEP], i32, tag="topot")
            iota_sb = pp.tile([P, B * S], i32, tag="iota")
            riota_i = pp.tile([P, B * S], i32, tag="riota_i")
            riota = pp.tile([P, B * S], f32, tag="riota")
            iota_f = pp.tile([P, B * S], f32, tag="iota_f")
            topot_f = pp.tile([P, B * NSTEP], f32, tag="topot_f")
            sero_sb = pp.tile([P, B * NSTEP], f32, tag="sero")
            ident = pp.tile([P, P], f32, tag="ident")
            w1a = pp.tile([P, 128], f32, tag="w1a")
            w1b = pp.tile([P, 128], f32, tag="w1b")
            w1c = pp.tile([P, 128], f32, tag="w1c")
            w2t = pp.tile([P, 128], f32, tag="w2t")
            wh1t = pp.tile([P, 128], f32, tag="wh1t")
            wh2t = pp.tile([P, 64], f32, tag="wh2t")
            w1f = pp.tile([P, 128], f32, tag="w1f")
            b1s = pp.tile([P, 1], f32, tag="b1s")
            b2s = pp.tile([P, 1], f32, tag="b2s")
            bh1s = pp.tile([P, 1], f32, tag="bh1s")
            t0 = pp.tile([P, 4], f32, tag="t0")

            # ---- setup ----
            import concourse.bass as bass
            make_identity(nc, ident[:])
            # offs: host-prepared [p, 4*i + b]; topot: DRAM[(b*128+p)*64+i] -> SBUF[p, 64*b+i]
            nc.sync.dma_start(out=offs_sb[:], in_=offs_d[:])
            nc.sync.dma_start(
                out=_v(topot_sb[:], 0, [[NSTEP, B], [1, NSTEP]]),
                in_=bass.AP(topot_d[:].tensor, 0,
                            [[NSTEP, P], [P * NSTEP, B], [1, NSTEP]]),
            )
            nc.sync.dma_start(out=w1a[:], in_=w1_d[0:128, :])
            nc.sync.dma_start(out=w1b[:], in_=w1_d[128:256, :])
            nc.sync.dma_start(out=w1c[0:68, :], in_=w1_d[256:324, :])
            nc.sync.dma_start(out=w1f[0:4, :], in_=w1_d[324:328, :])
            nc.sync.dma_start(out=w2t[:], in_=w2_d[:])
            nc.sync.dma_start(out=wh1t[:], in_=wh1_d[:])
            nc.sync.dma_start(out=wh2t[:], in_=wh2_d[:])
            nc.sync.dma_start(out=b1s[:], in_=b1_d[:].rearrange("(d o) -> d o", o=1))
            nc.sync.dma_start(out=b2s[:], in_=b2_d[:].rearrange("(d o) -> d o", o=1))
            nc.sync.dma_start(out=bh1s[:], in_=bh1_d[:].rearrange("(d o) -> d o", o=1))
            # constraints into each gather buffer's C_CONST columns
            for k in range(NG):
                nc.sync.dma_start(
                    out=_v(G[k][:], C_CONST, [[BW, B], [1, 4]]),
                    in_=bass.AP(cons_d[:].tensor, 0, [[4, P], [P * 4, B], [1, 4]]),
                )
            nc.vector.memset(qos[:], -3.0)
            nc.vector.memset(_v(C[0][:], 1, [[4, B]]), 1.0)   # avail
            nc.vector.memset(_v(C[0][:], 2, [[4, B]]), 3.0)   # thr
            nc.vector.memset(_v(C[0][:], 3, [[4, B]]), 1.0)   # rel
            nc.gpsimd.iota(iota_sb[:].rearrange("p (a b) -> p a b", a=B),
                           pattern=[[0, B], [1, S]], base=0, channel_multiplier=0)
            nc.gpsimd.iota(riota_i[:].rearrange("p (a b) -> p a b", a=B),
                           pattern=[[0, B], [-1, S]], base=S, channel_multiplier=0)
            nc.vector.tensor_copy(riota[:], riota_i[:])
            nc.vector.tensor_copy(iota_f[:], iota_sb[:])
            nc.vector.tensor_copy(topot_f[:], topot_sb[:])

            tm_flat = tm_d[:]

            def gather(i):
                k = i % NG
                for b in range(B):
                    nc.gpsimd.indirect_dma_start(
                        out=G[k][:, BW * b:BW * b + 384],
                        out_offset=None,
                        in_=tm_flat,
                        in_offset=IndirectOffsetOnAxis(
                            ap=offs_sb[:, B * i + b:B * i + b + 1], axis=0),
                    )

            for i in range(NG):
                gather(i)

            def phaseA(i):
                k = i % NG
                g = G[k]
                pc0 = pch.tile([P, 512], f32, tag="pc0", name="pc0")
                pc1 = pch.tile([P, 512], f32, tag="pc1", name="pc1")
                pc2 = pch.tile([P, 512], f32, tag="pc2", name="pc2")
                for b in range(B):
                    cb = BW * b
                    nc.tensor.transpose(out=pc0[:, P * b:P * (b + 1)],
                                        in_=g[:, cb + C_T:cb + C_T + 128], identity=ident[:])
                    nc.tensor.transpose(out=pc1[:, P * b:P * (b + 1)],
                                        in_=g[:, cb + C_T + 128:cb + C_T + 256], identity=ident[:])
                    nc.tensor.transpose(out=pc2[0:68, P * b:P * (b + 1)],
                                        in_=g[:, cb + 320:cb + 388], identity=ident[:])
                st0 = wp.tile([P, 512], f32, tag="st0", bufs=3, name="st0")
                st1 = wp.tile([P, 512], f32, tag="st1", bufs=3, name="st1")
                st2 = wp.tile([P, 512], f32, tag="st2", bufs=3, name="st2")
                nc.scalar.copy(out=st0[:], in_=pc0[:])
                nc.scalar.copy(out=st1[:], in_=pc1[:])
                nc.vector.tensor_copy(out=st2[0:68, :], in_=pc2[0:68, :])
                return st0, st1, st2

            def phaseB(st0, st1, st2):
                ph = pml.tile([P, 512], f32, tag="ph", bufs=2, name="ph")
                nc.tensor.matmul(ph[:], w1a[:], st0[:], start=True, stop=False)
                nc.tensor.matmul(ph[:], w1b[:], st1[:], start=False, stop=False)
                nc.tensor.matmul(ph[:], w1c[0:68, :], st2[0:68, :], start=False, stop=False)
                return ph

            def tail(i, ph):
                k = i % NG
                g = G[k]
                A, Cb = C[i % 2], C[(i + 1) % 2]

                # 1) rt = max_n task64 * qos  (gpsimd mul + DVE reduce)
                prod = wp.tile([P, B * S], f32, tag="prod")
                nc.gpsimd.tensor_tensor(
                    out=prod[:], in0=_v(g[:], C_T, [[BW, B], [1, S]]),
                    in1=qos[:], op=AOp.mult)
                rt_dst = _v(A[:], 0, [[4, B]])
                if i == 0:
                    nc.vector.tensor_reduce(
                        out=t0[:], in_=prod[:].rearrange("p (a b) -> p a b", a=B),
                        axis=AX.X, op=AOp.max)
                    nc.vector.tensor_scalar_add(out=rt_dst, in0=t0[:], scalar1=-3.0)
                else:
                    nc.vector.tensor_reduce(
                        out=rt_dst, in_=prod[:].rearrange("p (a b) -> p a b", a=B),
                        axis=AX.X, op=AOp.max)

                # 4) feat contribution appended to the W1 PSUM group
                prt = pqv.tile([4, 512], f32, tag="prt", name="prt")
                for b in range(B):
                    nc.tensor.transpose(out=prt[0:4, P * b:P * (b + 1)],
                                        in_=A[:, 4 * b:4 * b + 4],
                                        identity=ident[:])
                rts = wp.tile([4, 512], f32, tag="rts", name="rts")
                nc.scalar.copy(out=rts[0:4, :], in_=prt[0:4, :])
                nc.tensor.matmul(ph[:], w1f[0:4, :], rts[0:4, :], start=False, stop=True)
                hs = wp.tile([P, 512], f32, tag="hs")
                nc.scalar.activation(out=hs[:], in_=ph[:], func=AF.Silu, bias=b1s[:])

                pe = pml.tile([P, 512], f32, tag="pe")
                nc.tensor.matmul(pe[:], w2t[:], hs[:], start=True, stop=True)
                xs = wp.tile([P, 512], f32, tag="xs")
                nc.scalar.activation(out=xs[:], in_=pe[:], func=AF.Silu, bias=b2s[:])

                ph2 = pml.tile([P, 512], f32, tag="pe")
                nc.tensor.matmul(ph2[:], wh1t[:], xs[:], start=True, stop=True)
                h2s = wp.tile([P, 512], f32, tag="h2s")
                nc.scalar.activation(out=h2s[:], in_=ph2[:], func=AF.Silu, bias=bh1s[:])

                pqvt = pqv.tile([P, B * S], f32, tag="pqv")
                for b in range(B):
                    nc.tensor.matmul(pqvt[:, S * b:S * (b + 1)],
                                     h2s[:, P * b:P * (b + 1)], wh2t[:],
                                     start=True, stop=True)

                # 5) masked argmax (additive mask+bias already in G's M cols)
                qvm = wp.tile([P, B * S], f32, tag="qvm")
                nc.vector.tensor_tensor(out=qvm[:], in0=pqvt[:],
                                        in1=_v(g[:], C_M, [[BW, B], [1, S]]), op=AOp.add)
                mx = wp.tile([P, B], f32, tag="mx")
                nc.vector.tensor_reduce(out=mx[:],
                                        in_=qvm[:].rearrange("p (a b) -> p a b", a=B),
                                        axis=AX.X, op=AOp.max)
                oh = wp.tile([P, B * S], f32, tag="oh")
                nc.vector.tensor_tensor(out=oh[:], in0=qvm[:],
                                        in1=mx[:].to_broadcast([P, B, S]), op=AOp.is_equal)
                serv = wp.tile([P, B * S], f32, tag="serv")
                nc.vector.tensor_tensor(out=serv[:], in0=oh[:], in1=riota[:], op=AOp.mult)
                nc.vector.tensor_reduce(
                    out=_v(sero_sb[:], B * i, [[1, B]]),
                    in_=serv[:].rearrange("p (a b) -> p a b", a=B),
                    axis=AX.X, op=AOp.max)

                # 6) sq = service features at argmax: g-mul (gpsimd) + reduce (DVE)
                gm = wp.tile([P, B * S * 4], f32, tag="gm")
                nc.gpsimd.tensor_tensor(
                    out=gm[:], in0=_v(g[:], C_T + 64, [[BW, B], [4, S], [1, 4]]),
                    in1=_v(oh[:], 0, [[S, B], [1, S], [0, 4]]), op=AOp.mult)
                sq = wp.tile([P, 16], f32, tag="sq")
                nc.vector.tensor_reduce(
                    out=sq[:], in_=_v(gm[:], 0, [[S * 4, B], [1, 4], [4, S]]),
                    axis=AX.X, op=AOp.add)

                # 7) carry updates into Cb
                nc.vector.tensor_tensor(out=_v(Cb[:], 0, [[4, B]]),
                                        in0=_v(sq[:], 0, [[4, B]]),
                                        in1=_v(A[:], 0, [[4, B]]), op=AOp.add)
                nc.vector.tensor_tensor(out=_v(Cb[:], 1, [[4, B], [2, 2]]),
                                        in0=_v(sq[:], 1, [[4, B], [2, 2]]),
                                        in1=_v(A[:], 1, [[4, B], [2, 2]]), op=AOp.mult)
                nc.vector.tensor_tensor(out=_v(Cb[:], 2, [[4, B]]),
                                        in0=_v(sq[:], 2, [[4, B]]),
                                        in1=_v(A[:], 2, [[4, B]]), op=AOp.min)

                # 8) qos scatter: qos[q, topo] = new_rt
                oht = wp.tile([P, B * S], i32, tag="oht")
                nc.vector.tensor_tensor(
                    out=oht[:], in0=iota_sb[:],
                    in1=_v(topot_sb[:], i, [[NSTEP, B], [0, S]]), op=AOp.is_equal)
                nc.vector.copy_predicated(
                    out=qos[:].rearrange("p (a b) -> p a b", a=B),
                    mask=oht[:].rearrange("p (a b) -> p a b", a=B),
                    data=_v(Cb[:], 0, [[4, B], [0, S]]))

            BATCH = 2
            for j in range(0, NSTEP, BATCH):
                sts = [phaseA(i) for i in range(j, j + BATCH)]
                phs = [phaseB(*s) for s in sts]
                for bi, i in enumerate(range(j, j + BATCH)):
                    tail(i, phs[bi])
                    if i + NG < NSTEP:
                        gather(i + NG)

            nc.sync.dma_start(out=sero_d[:], in_=sero_sb[:])

    nc.compile()
    return nc


def _host_prep(tasks, constraints, masks, topologicals, bh2):
    """Build fused TM table, reversed topo, gather offsets; per-core shards."""
    Qf = tasks.shape[0]
    ncores = Qf // QL
    M = (masks.astype(np.float32) - 1.0) * 1e9 + bh2[None, None, :].astype(np.float32)
    tm = np.concatenate([M, tasks], axis=2)                     # [Q, 64, 384]
    topot = topologicals[:, ::-1].astype(np.int32)              # [Q, 64] reversed
    ql = np.arange(Qf, dtype=np.int32) % QL
    offs_qi = ql[:, None] * 64 + topot                          # [Q, 64]
    # per-core [p, 4*i + b] layout for contiguous per-step offset slices
    offs = offs_qi.reshape(ncores, B, P, NSTEP).transpose(0, 2, 3, 1)  # [c, p, i, b]
    offs = np.ascontiguousarray(offs.reshape(ncores, P, NSTEP * B))
    return tm, topot, offs


def kernel(tasks, constraints, masks, topologicals,
           W1, b1, W2, b2, Wh1, bh1, Wh2, bh2):
    from concourse.bass_utils import run_bass_kernel_spmd

    tasks = np.asarray(tasks, dtype=np.float32)
    constraints = np.asarray(constraints, dtype=np.float32)
    masks = np.asarray(masks)
    topologicals = np.asarray(topologicals)
    W1 = np.asarray(W1, dtype=np.float32)
    W2 = np.asarray(W2, dtype=np.float32)
    Wh1 = np.asarray(Wh1, dtype=np.float32)
    Wh2 = np.asarray(Wh2, dtype=np.float32)
    b1 = np.asarray(b1, dtype=np.float32)
    b2 = np.asarray(b2, dtype=np.float32)
    bh1 = np.asarray(bh1, dtype=np.float32)
    bh2 = np.asarray(bh2, dtype=np.float32)

    tm, topot, offs = _host_prep(tasks, constraints, masks, topologicals, bh2)

    if "nc" not in _cached:
        _cached["nc"] = build_program()
    nc = _cached["nc"]

    in_maps = []
    for c in range(NC):
        sl = slice(c * QL, (c + 1) * QL)
        in_maps.append({
            "tm": np.ascontiguousarray(tm[sl].reshape(QL * 64, 384)),
            "offs": offs[c],
            "topot": np.ascontiguousarray(topot[sl].reshape(-1)),
            "cons": np.ascontiguousarray(constraints[sl].reshape(-1)),
            "w1": W1, "w2": W2, "wh1": Wh1, "wh2": Wh2,
            "b1": b1, "b2": b2, "bh1": bh1,
        })

    trace = bool(int(os.environ.get("KERNEL_TRACE", "0")))
    res = run_bass_kernel_spmd(nc, in_maps, core_ids=list(range(NC)), trace=trace)
    _cached["last_result"] = res

    ret = np.zeros((tasks.shape[0], 64), np.float32)
    rows = np.arange(tasks.shape[0])
    for c in range(NC):
        sero = res.results[c]["sero"]                 # [128, 4*64]
        ser = 64.0 - sero.reshape(P, NSTEP, B)        # [p, i, b]
        ser = ser.transpose(2, 0, 1).reshape(QL, NSTEP)  # [q_local, i]
        sl = slice(c * QL, (c + 1) * QL)
        for i in range(NSTEP):
            np.add.at(ret, (rows[sl], topot[sl, i]), ser[:, i])
    return ret.astype(np.int16)
